# revision 1
# baseline (speedup 1.0000x reference)
"""Talking-heads attention Trainium2 kernel (8-core data-parallel over batch).

Reference computation (per batch item):
    q = x @ Wq ; k,v = x @ Wkv          (h=12 heads, d=64)
    S[h] = (q_h k_h^T) * d**-0.5
    S'[g] = sum_h mix_pre[h,g] S[h]     (talking-heads pre-softmax)
    P = softmax_j(S')
    P''[h] = sum_g mix_post[g,h] P[g]   (talking-heads post-softmax)
    out = concat_h(P''[h] @ v_h) @ Wo + bo

Design:
  * fp16 operands on the PE (fp32 PSUM accumulate); exp/softmax math fp32.
  * Logits are materialized partition-interleaved: 96 rows = 8 query rows x
    12 heads (each block padded to 128 for the DMA-transpose xbar, which
    folds output rows in fixed 128 blocks).  Both talking-heads mixes are
    then single 96x96 matmuls (block-structured mix matrices) per 512-wide
    j slab.
  * Softmax needs no max subtraction (logits are O(20)); exp runs on
    ScalarE with the fused per-partition sum (accum_out).  The 1/sum
    normalization is folded into the post-mix matrix (f32r, scaled per
    partition row by the DVE), costing one tiny [96,96] op per row group.
  * The partition interleave and the j-major transpose for attn@V use the
    DMA transpose crossbar (SBUF->SBUF, 2-byte), keeping VectorE/ScalarE
    free for the mandatory PSUM->SBUF evictions.
"""

import os
import numpy as np
import ml_dtypes

import concourse.bass as bass
import concourse.bacc as bacc
import concourse.mybir as mybir
import concourse.tile as tile
from concourse.bass_utils import run_bass_kernel_spmd
from contextlib import ExitStack

BF16_NP = ml_dtypes.bfloat16
FP16_NP = np.float16

F32 = mybir.dt.float32
BF16 = mybir.dt.bfloat16
FP16 = mybir.dt.float16
F32R = mybir.dt.float32r

# problem shape (hardcoded per contest rules)
B_TOTAL = 16
N_CORES = 8
B = B_TOTAL // N_CORES  # batch items per core
N = 1024                # sequence length
DIM = 768               # model dim
H = 12                  # heads
DH = 64                 # head dim
HC = H * DH             # 768
NK = DIM // 128         # 6 contraction tiles of 128
SCALE = DH ** -0.5

ILOC = 8                # query rows per row-group
GRP = ILOC * H          # 96 live partitions per row-group
CH = 128                # query rows per chunk
NIG = CH // ILOC        # 16 row-groups per chunk
NCH = N // CH           # 8 chunks per batch item
NJB = N // 128          # 8 key blocks
NCHP = N // 256         # 4 QK chunk pairs


def _copy(nc, idx, out, in_, scale=None):
    if idx % 2 == 0:
        if scale is None:
            nc.vector.tensor_copy(out, in_)
        else:
            nc.vector.tensor_scalar_mul(out, in_, scale)
    else:
        if scale is None:
            nc.scalar.copy(out, in_)
        else:
            nc.scalar.mul(out, in_, scale)


def build_program(debug_taps=(), reps=1):
    nc = bacc.Bacc(
        "TRN2",
        target_bir_lowering=False,
        debug=False,
        num_devices=N_CORES,
    )
    taps = {}
    for name, shape, dt in debug_taps:
        taps[name] = nc.declare_dram_parameter(name, list(shape), dt,
                                               isOutput=True)

    # all inputs pre-cast to fp16 host-side; bd matrices pre-built host-side
    x_d = nc.declare_dram_parameter("xb", [B, N, DIM], FP16, isOutput=False)
    wq_d = nc.declare_dram_parameter("wqb", [DIM, HC], FP16, isOutput=False)
    wk_d = nc.declare_dram_parameter("wkb", [DIM, HC], FP16, isOutput=False)
    wv_d = nc.declare_dram_parameter("wvb", [DIM, HC], FP16, isOutput=False)
    wo_d = nc.declare_dram_parameter("wob", [HC, DIM], FP16, isOutput=False)
    bdpre_d = nc.declare_dram_parameter("bdpre", [GRP, GRP], FP16, isOutput=False)
    bdpat_d = nc.declare_dram_parameter("bdpat", [GRP, GRP], F32, isOutput=False)
    bo_d = nc.declare_dram_parameter("bob", [1, DIM], FP16, isOutput=False)
    y_d = nc.declare_dram_parameter("y", [B, N, DIM], F32, isOutput=True)

    cc = [0]  # copy-engine alternation counter

    def nxt():
        cc[0] += 1
        return cc[0]

    with tile.TileContext(nc) as tc:
        with ExitStack() as ctx:
            persist = ctx.enter_context(tc.tile_pool(name="persist", bufs=1))
            sT_pool = ctx.enter_context(tc.tile_pool(name="sT", bufs=1))
            sInt_pool = ctx.enter_context(tc.tile_pool(name="sInt", bufs=1))
            e_pool = ctx.enter_context(tc.tile_pool(name="epool", bufs=2))
            p2_pool = ctx.enter_context(tc.tile_pool(name="p2pool", bufs=2))
            pt_pool = ctx.enter_context(tc.tile_pool(name="ptpool", bufs=1))
            small = ctx.enter_context(tc.tile_pool(name="small", bufs=4))
            y_pool = ctx.enter_context(tc.tile_pool(name="ypool", bufs=2))
            xbf_pool = ctx.enter_context(tc.tile_pool(name="xbf", bufs=1))

            ps_qkav = ctx.enter_context(
                tc.tile_pool(name="ps_qkav", bufs=2, space="PSUM"))
            ps_mix = ctx.enter_context(
                tc.tile_pool(name="ps_mix", bufs=2, space="PSUM"))
            ps_proj = ctx.enter_context(
                tc.tile_pool(name="ps_proj", bufs=1, space="PSUM"))
            ps_y = ctx.enter_context(
                tc.tile_pool(name="ps_y", bufs=1, space="PSUM"))

            # ---------------- persistent constants ----------------
            wo_t = persist.tile([128, NK, DIM], FP16, tag="wo")
            for kt in range(NK):
                nc.sync.dma_start(out=wo_t[:, kt, :],
                                  in_=wo_d[kt * 128:(kt + 1) * 128, :])
            bo_t = persist.tile([1, DIM], FP16, tag="bo")
            nc.sync.dma_start(out=bo_t[:], in_=bo_d[:, :])
            ones_t = persist.tile([1, CH], FP16, tag="ones")
            nc.vector.memset(ones_t[:], 1.0)
            bd_pre = persist.tile([GRP, GRP], FP16, tag="bdpre")
            nc.sync.dma_start(out=bd_pre[:], in_=bdpre_d[:, :])
            bd_pat = persist.tile([GRP, GRP], F32, tag="bdpat")
            nc.sync.dma_start(out=bd_pat[:], in_=bdpat_d[:, :])

            qT = persist.tile([128, NK, N], FP16, tag="qT")
            kT = persist.tile([128, NK, N], FP16, tag="kT")
            v_t = persist.tile([128, NJB, HC], FP16, tag="v")
            oaT = persist.tile([128, NK, N], FP16, tag="oaT")

            for rep in range(reps):
             for b in range(B):
                # qkv weights live in the sT slot; x^T lives in the sInt slot
                w3 = sT_pool.tile([128, 3, NK, HC], FP16, tag="sT")
                for kt in range(NK):
                    nc.sync.dma_start(out=w3[:, 0, kt, :],
                                      in_=wq_d[kt * 128:(kt + 1) * 128, :])
                    nc.sync.dma_start(out=w3[:, 1, kt, :],
                                      in_=wk_d[kt * 128:(kt + 1) * 128, :])
                    nc.sync.dma_start(out=w3[:, 2, kt, :],
                                      in_=wv_d[kt * 128:(kt + 1) * 128, :])
                xT = sInt_pool.tile([128, NK, N], FP16, tag="sInt")

                # ---------------- x load + transpose ----------------
                for ib in range(NCH):
                    x_bf = xbf_pool.tile([128, DIM], FP16, tag="xbf")
                    nc.sync.dma_start(
                        out=x_bf[:],
                        in_=x_d[b, ib * 128:(ib + 1) * 128, :])
                    nc.sync.dma_start_transpose(
                        out=xT[:, :, ib * 128:(ib + 1) * 128],
                        in_=x_bf[:])

                # ---------------- projections ----------------
                for wi, dst, scl in ((0, qT, SCALE), (1, kT, None)):
                    for ot in range(NK):        # output hc tile
                        for ic in range(2):     # i 512-slab
                            ps = ps_proj.tile([128, 512], F32, tag="proj")
                            for kt in range(NK):
                                nc.tensor.matmul(
                                    ps[:],
                                    lhsT=w3[:, wi, kt, ot * 128:(ot + 1) * 128],
                                    rhs=xT[:, kt, ic * 512:(ic + 1) * 512],
                                    start=(kt == 0), stop=(kt == NK - 1))
                            _copy(nc, nxt(), dst[:, ot, ic * 512:(ic + 1) * 512],
                                  ps[:], scale=scl)
                for jb in range(NJB):
                    for nh in range(2):         # v cols 0:512, 512:768
                        nw = 512 if nh == 0 else 256
                        ps = ps_proj.tile([128, 512], F32, tag="proj")
                        for kt in range(NK):
                            nc.tensor.matmul(
                                ps[:, 0:nw],
                                lhsT=xT[:, kt, jb * 128:(jb + 1) * 128],
                                rhs=w3[:, 2, kt, nh * 512:nh * 512 + nw],
                                start=(kt == 0), stop=(kt == NK - 1))
                        _copy(nc, nxt(), v_t[:, jb, nh * 512:nh * 512 + nw],
                              ps[:, 0:nw])

                if b == 0 and rep == 0:
                    for nm, src in (("dbg_qT", qT), ("dbg_kT", kT),
                                    ("dbg_v", v_t), ("dbg_xT", xT)):
                        if nm in taps:
                            nc.sync.dma_start(out=taps[nm][:], in_=src[:])

                # ---------------- attention ----------------
                for chp in range(NCHP):
                    # QK^T for a 256-row chunk pair, j on partitions.
                    # sT free layout: igrp blocks of 128 cols (col =
                    # (i%8)*12+h, 96:128 pad) -- xbar folds rows by 128.
                    sT = sT_pool.tile([128, NJB, 2 * NIG, 128], FP16, tag="sT")
                    mm_i = 0
                    ps = None
                    for h in range(H):
                        ht, hr = divmod(h, 2)
                        for jb in range(NJB):
                            slot = mm_i % 2
                            if slot == 0:
                                ps = ps_qkav.tile([128, 512], F32, tag="qkav")
                            nc.tensor.matmul(
                                ps[:, slot * 256:(slot + 1) * 256],
                                lhsT=kT[hr * 64:(hr + 1) * 64, ht,
                                        jb * 128:(jb + 1) * 128],
                                rhs=qT[hr * 64:(hr + 1) * 64, ht,
                                       chp * 256:(chp + 1) * 256],
                                start=True, stop=True)
                            mm_i += 1
                            if slot == 1:
                                # psum holds (h, jb-1) then (h, jb), each
                                # [128 j, 256 i] -> scatter into sT blocks
                                src = ps[:].rearrange(
                                    "p (s a c) -> p s a c", s=2, c=ILOC)
                                dst = sT[:, jb - 1:jb + 1, :, h:h + 96:12]
                                _copy(nc, nxt(), dst, src)

                    if b == 0 and chp == 0 and rep == 0 and "dbg_sT" in taps:
                        nc.sync.dma_start(out=taps["dbg_sT"][:], in_=sT[:])

                    for half in range(2):
                        ch = 2 * chp + half
                        # interleave: S_int[(i%8)*12+h, i//8, j] (96:128 junk)
                        sInt = sInt_pool.tile([128, NIG, N], FP16, tag="sInt")
                        for jb in range(NJB):
                            for _dup in range(1 + int(os.environ.get("KDUP1", "0"))):
                                nc.sync.dma_start_transpose(
                                    out=sInt[:, :, jb * 128:(jb + 1) * 128],
                                    in_=sT[:, jb, NIG * half:NIG * (half + 1), :])
                        if (b == 0 and ch == 0 and rep == 0
                                and "dbg_sInt" in taps):
                            nc.sync.dma_start(out=taps["dbg_sInt"][:],
                                              in_=sInt[:])

                        # per row-group: premix -> exp+sum -> postmix -> T
                        pT = pt_pool.tile([128, NJB, NIG, GRP], FP16, tag="pT")
                        for ig in range(NIG):
                            psm = ps_mix.tile([GRP, N], F32, tag="mix")
                            for jj in range(2):
                                nc.tensor.matmul(
                                    psm[:, jj * 512:(jj + 1) * 512],
                                    lhsT=bd_pre[:],
                                    rhs=sInt[0:GRP, ig,
                                             jj * 512:(jj + 1) * 512],
                                    start=True, stop=True)
                            e_sb = e_pool.tile([GRP, N], F32R, tag="esb")
                            ssum = small.tile([GRP, 1], F32, tag="ssum")
                            nc.scalar.activation(
                                e_sb[:], psm[:],
                                mybir.ActivationFunctionType.Exp,
                                accum_out=ssum[:])
                            recip = small.tile([GRP, 1], F32, tag="recip")
                            nc.vector.reciprocal(recip[:], ssum[:])
                            bd_ps = small.tile([GRP, GRP], F32R, tag="bdps")
                            nc.vector.tensor_scalar_mul(bd_ps[:], bd_pat[:],
                                                        recip[:])
                            if (b == 0 and ch == 0 and ig == 0 and rep == 0
                                    and "dbg_E" in taps):
                                nc.sync.dma_start(out=taps["dbg_E"][:],
                                                  in_=e_sb[:].bitcast(F32))
                                if "dbg_sums" in taps:
                                    nc.sync.dma_start(out=taps["dbg_sums"][:],
                                                      in_=ssum[:])
                            psp = ps_mix.tile([GRP, N], F32, tag="mix")
                            for jj in range(2):
                                nc.tensor.matmul(
                                    psp[:, jj * 512:(jj + 1) * 512],
                                    lhsT=bd_ps[:],
                                    rhs=e_sb[:, jj * 512:(jj + 1) * 512],
                                    start=True, stop=True)
                            p2 = p2_pool.tile([GRP, N], FP16, tag="p2")
                            _copy(nc, nxt(), p2[:], psp[:])
                            # transpose to P_T[j, (h*8+i_local)]
                            nc.sync.dma_start_transpose(
                                out=pT[:, :, ig, :], in_=p2[:])

                        if (b == 0 and ch == 0 and rep == 0
                                and "dbg_pT" in taps):
                            nc.sync.dma_start(out=taps["dbg_pT"][:], in_=pT[:])

                        # attn @ V (two heads share one psum, column tiling)
                        for hp in range(H // 2):
                            psa = ps_qkav.tile([128, 512], F32, tag="qkav")
                            for hh in range(2):
                                h = 2 * hp + hh
                                for jb in range(NJB):
                                    nc.tensor.matmul(
                                        psa[64 * hh:64 * (hh + 1), 0:128],
                                        lhsT=v_t[:, jb, h * 64:(h + 1) * 64],
                                        rhs=pT[:, jb, :, 8 * h:8 * h + 8],
                                        start=(jb == 0), stop=(jb == NJB - 1),
                                        tile_position=(0, 64 * hh))
                            _copy(nc, nxt(),
                                  oaT[:, hp, ch * 128:(ch + 1) * 128],
                                  psa[:, 0:128])

                        if (b == 0 and ch == NCH - 1 and rep == 0
                                and "dbg_oaT" in taps):
                            nc.sync.dma_start(out=taps["dbg_oaT"][:],
                                              in_=oaT[:])

                        # output projection + bias (two column halves)
                        y_sb = y_pool.tile([128, DIM], F32, tag="ysb")
                        for nh in range(2):
                            nw = 512 if nh == 0 else 256
                            psy = ps_y.tile([128, 512], F32, tag="yps")
                            for kt in range(NK):
                                nc.tensor.matmul(
                                    psy[:, 0:nw],
                                    lhsT=oaT[:, kt, ch * 128:(ch + 1) * 128],
                                    rhs=wo_t[:, kt, nh * 512:nh * 512 + nw],
                                    start=(kt == 0), stop=False)
                            nc.tensor.matmul(
                                psy[:, 0:nw], lhsT=ones_t[:],
                                rhs=bo_t[:, nh * 512:nh * 512 + nw],
                                start=False, stop=True)
                            _copy(nc, nxt(),
                                  y_sb[:, nh * 512:nh * 512 + nw],
                                  psy[:, 0:nw])
                        nc.sync.dma_start(
                            out=y_d[b, ch * 128:(ch + 1) * 128, :],
                            in_=y_sb[:])

    nc.compile()
    return nc


def host_prep(inputs):
    """Pre-cast weights to fp16 and build the block-structured mix matrices."""
    mix_pre = np.asarray(inputs["mix_pre"], dtype=np.float32)
    mix_post = np.asarray(inputs["mix_post"], dtype=np.float32)
    # bd_pre[(i*12+h), (i*12+g)] = mix_pre[h, g]
    bd_pre = np.zeros((GRP, GRP), dtype=np.float32)
    # bd_pat[(i*12+g), (h*8+i)] = mix_post[g, h]
    bd_pat = np.zeros((GRP, GRP), dtype=np.float32)
    for i in range(ILOC):
        bd_pre[12 * i:12 * i + 12, 12 * i:12 * i + 12] = mix_pre
        for h in range(H):
            bd_pat[12 * i:12 * i + 12, h * ILOC + i] = mix_post[:, h]
    wkv = np.asarray(inputs["Wkv"], dtype=np.float32)
    common = {
        "wqb": np.asarray(inputs["Wq"], dtype=np.float32).astype(FP16_NP),
        "wkb": np.ascontiguousarray(wkv[:, :HC]).astype(FP16_NP),
        "wvb": np.ascontiguousarray(wkv[:, HC:]).astype(FP16_NP),
        "wob": np.asarray(inputs["Wo"], dtype=np.float32).astype(FP16_NP),
        "bob": np.asarray(inputs["bo"], dtype=np.float32).reshape(1, DIM)
               .astype(FP16_NP),
        "bdpre": bd_pre.astype(FP16_NP),
        "bdpat": bd_pat,
    }
    return common


def kernel(**inputs):
    x = np.asarray(inputs["x"], dtype=np.float32).astype(FP16_NP)
    common = host_prep(inputs)
    nc = build_program()
    in_maps = []
    for c in range(N_CORES):
        m = dict(common)
        m["xb"] = np.ascontiguousarray(x[c * B:(c + 1) * B])
        in_maps.append(m)
    res = run_bass_kernel_spmd(nc, in_maps, list(range(N_CORES)))
    out = np.concatenate([res.results[c]["y"] for c in range(N_CORES)], axis=0)
    return out.astype(np.float32)


if __name__ == "__main__":
    rng = np.random.default_rng(0)
    ins = {
        "x": rng.standard_normal((B_TOTAL, N, DIM), dtype=np.float32),
        "Wq": rng.standard_normal((DIM, HC), dtype=np.float32) * DIM ** -0.5,
        "Wkv": rng.standard_normal((DIM, 2 * HC), dtype=np.float32) * DIM ** -0.5,
        "mix_pre": rng.standard_normal((H, H), dtype=np.float32),
        "mix_post": rng.standard_normal((H, H), dtype=np.float32),
        "Wo": rng.standard_normal((HC, DIM), dtype=np.float32) * HC ** -0.5,
        "bo": np.zeros(DIM, dtype=np.float32),
    }
    y = kernel(**ins)
    print("kernel output", y.shape, y.dtype, float(np.abs(y).max()))



# revision 3
# speedup vs baseline: 14.1492x; 14.1492x over previous
"""Talking-heads attention Trainium2 kernel, v2 (8-core data-parallel over batch).

Same math as the baseline kernel (see reference.py) but restructured to
minimize instruction count and data movement:

  * AV (attn @ V) runs at free-dim 256 (one chunk-pair) instead of 128:
    384 -> 96 matmuls per chunk-pair... (12h x 8jb at free 256).
  * All DMA transposes are batched: one xbar call per sT half (was 8),
    one per 2 row-groups for pT (was 1 per group), one per x quarter.
  * QK PSUM evictions write 8 key-blocks per DVE scatter (was 2).
  * Weights arrive in one packed dram tensor (single DMA); x in 4 DMAs.
  * Scale folded into Wq host-side; output bias via two ones-matmuls
    per 128-row chunk; y stored fp16 and upcast host-side.

Layouts (per core, B=2 batch items, N=1024, 12 heads x 64):
  xT   [128, 8, 6, 128]   x^T: [dim-in-tile, i-block, k-tile, i-in-block]
  qT/kT[128, 6, 1024]     per-head-column projections (d on partitions)
  v_t  [128, 8, 768]      V with j on partitions
  sT   [128, 2, 8, 16, 128]  S^T per chunk-pair: [j-in-blk, half, jb, ig, (iloc*12+h pad 128)]
  sInt [128, 8, 16, 128]  premix layout: [(iloc*12+h) pad, jb, ig, j-in-blk]
  p2   [96, 2, 1024]      postmixed P'' rows (h*8+iloc) for 2 row-groups
  pT   [128, 32, 8, 96]   P''^T per chunk-pair: [j-in-blk, ig, jb, (h*8+iloc)]
  oaT  [128, 6, 256]      attention out: [(h%2)*64+d, head-pair, i]
"""

import numpy as np
import ml_dtypes

import concourse.bass as bass
import concourse.bacc as bacc
import concourse.mybir as mybir
import concourse.tile as tile
from concourse.bass_utils import run_bass_kernel_spmd
from contextlib import ExitStack

FP16_NP = np.float16

F32 = mybir.dt.float32
FP16 = mybir.dt.float16
F32R = mybir.dt.float32r

B_TOTAL = 16
N_CORES = 8
B = B_TOTAL // N_CORES  # 2 batch items per core
N = 1024
DIM = 768
H = 12
DH = 64
HC = H * DH             # 768
NK = DIM // 128         # 6 contraction tiles
SCALE = DH ** -0.5

ILOC = 8                # query rows per row-group
GRP = ILOC * H          # 96 live partitions per row-group
NJB = N // 128          # 8 key blocks
NCP = N // 256          # 4 chunk-pairs per batch item (256 rows each)
NIG = 32                # row-groups per chunk-pair


def build_program(debug_taps=(), reps=1):
    nc = bacc.Bacc(
        "TRN2",
        target_bir_lowering=False,
        debug=False,
        num_devices=N_CORES,
    )
    taps = {}
    for name, shape, dt in debug_taps:
        taps[name] = nc.declare_dram_parameter(name, list(shape), dt,
                                               isOutput=True)

    x_d = nc.declare_dram_parameter("xb", [B, N, DIM], FP16, isOutput=False)
    w3_d = nc.declare_dram_parameter("w3b", [DIM, 3 * HC], FP16, isOutput=False)
    wo_d = nc.declare_dram_parameter("wob", [HC, DIM], FP16, isOutput=False)
    bo_d = nc.declare_dram_parameter("bob", [1, DIM], FP16, isOutput=False)
    bdpre_d = nc.declare_dram_parameter("bdpre", [GRP, GRP], FP16, isOutput=False)
    bdpat_d = nc.declare_dram_parameter("bdpat", [GRP, GRP], F32, isOutput=False)
    y_d = nc.declare_dram_parameter("y", [B, N, DIM], FP16, isOutput=True)

    with tile.TileContext(nc) as tc:
        with ExitStack() as ctx:
            persist = ctx.enter_context(tc.tile_pool(name="persist", bufs=1))
            work = ctx.enter_context(tc.tile_pool(name="work", bufs=1))
            small = ctx.enter_context(tc.tile_pool(name="small", bufs=2))
            psA = ctx.enter_context(
                tc.tile_pool(name="psA", bufs=1, space="PSUM"))
            psB = ctx.enter_context(
                tc.tile_pool(name="psB", bufs=1, space="PSUM"))

            # ---------------- persistent constants ----------------
            wo_t = persist.tile([128, NK, DIM], FP16, tag="wo")
            nc.sync.dma_start(
                out=wo_t[:],
                in_=wo_d[:, :].rearrange("(k p) c -> p k c", p=128))
            bo_t = persist.tile([1, DIM], FP16, tag="bo")
            nc.sync.dma_start(out=bo_t[:], in_=bo_d[:, :])
            ones_t = persist.tile([1, 128], FP16, tag="ones")
            nc.vector.memset(ones_t[:], 1.0)
            bd_pre = persist.tile([GRP, GRP], FP16, tag="bdpre")
            nc.sync.dma_start(out=bd_pre[:], in_=bdpre_d[:, :])
            bd_pat = persist.tile([GRP, GRP], F32, tag="bdpat")
            nc.sync.dma_start(out=bd_pat[:], in_=bdpat_d[:, :])

            qT = persist.tile([128, NK, N], FP16, tag="qT")
            kT = persist.tile([128, NK, N], FP16, tag="kT")
            v_t = persist.tile([128, NJB, HC], FP16, tag="v")
            # S^T staging; columns 96:128 of each block stay zero forever.
            sT = persist.tile([128, 2, NJB, 16, 128], FP16, tag="sT")
            nc.vector.memset(sT[:], 0.0)

            for rep in range(reps):
             for b in range(B):
                # -------- x load + transpose (4 quarters) --------
                xT = work.tile([128, 8, NK, 128], FP16, tag="sInt")
                for q8 in range(8):
                    x_bf = work.tile([128, DIM], FP16, tag="xbf")
                    nc.sync.dma_start(
                        out=x_bf[:],
                        in_=x_d[b, q8 * 128:(q8 + 1) * 128, :])
                    nc.sync.dma_start_transpose(
                        out=xT[:, q8, :, :], in_=x_bf[:])

                # -------- packed QKV weights (single DMA) --------
                w3 = work.tile([128, NK, 3 * HC], FP16, tag="big")
                nc.sync.dma_start(
                    out=w3[:],
                    in_=w3_d[:, :].rearrange("(k p) c -> p k c", p=128))

                # -------- Q, K projections --------
                for ot in range(12):     # 0..5 -> qT, 6..11 -> kT
                    dst = qT if ot < 6 else kT
                    pp = psA.tile([128, 2, 512], F32, tag="A")
                    for kt in range(NK):
                        for ic in range(2):
                            nc.tensor.matmul(
                                pp[:, ic, :],
                                lhsT=w3[:, kt, ot * 128:(ot + 1) * 128],
                                rhs=xT[:, 4 * ic:4 * ic + 4, kt, :],
                                start=(kt == 0), stop=(kt == NK - 1))
                    nc.vector.tensor_copy(dst[:, ot % 6, :], pp[:])

                # -------- V projection --------
                for jbl in range(NJB):
                    pv = psA.tile([128, 2, 512], F32, tag="A")
                    for kt in range(NK):
                        for nh in range(2):
                            nc.tensor.matmul(
                                pv[:, nh, 0:384],
                                lhsT=xT[:, jbl, kt, :],
                                rhs=w3[:, kt,
                                       2 * HC + nh * 384:2 * HC + nh * 384 + 384],
                                start=(kt == 0), stop=(kt == NK - 1))
                    nc.vector.tensor_copy(
                        v_t[:, jbl, :].rearrange("p (a c) -> p a c", a=2),
                        pv[:, :, 0:384])

                if b == 0 and rep == 0:
                    for nm, src in (("dbg_qT", qT), ("dbg_kT", kT),
                                    ("dbg_v", v_t), ("dbg_xT", xT)):
                        if nm in taps:
                            nc.sync.dma_start(out=taps[nm][:], in_=src[:])

                # ---------------- attention ----------------
                for cp in range(NCP):
                    # QK^T: per head, 8 key-blocks into one 4-bank tile.
                    for h in range(H):
                        ht, hr = divmod(h, 2)
                        qk = psA.tile([128, NJB, 256], F32, tag="A")
                        for jb in range(NJB):
                            nc.tensor.matmul(
                                qk[:, jb, :],
                                lhsT=kT[hr * 64:(hr + 1) * 64, ht,
                                        jb * 128:(jb + 1) * 128],
                                rhs=qT[hr * 64:(hr + 1) * 64, ht,
                                       cp * 256:(cp + 1) * 256],
                                start=(jb % 2 == 0), stop=(jb % 2 == 1))
                        # scatter into sT interleaved columns (4D APs)
                        for hf in range(2):
                            nc.vector.tensor_copy(
                                sT[:, hf, :, :, h:h + GRP:H],
                                qk[:, :, 128 * hf:128 * hf + 128]
                                .rearrange("p j (a c) -> p j a c", c=ILOC))

                    if b == 0 and cp == 0 and rep == 0 and "dbg_sT" in taps:
                        nc.sync.dma_start(out=taps["dbg_sT"][:], in_=sT[:])

                    pT = work.tile([128, NIG, NJB, GRP], FP16, tag="big")
                    for half in range(2):
                        sInt = work.tile([128, NJB, 16, 128], FP16,
                                         tag="sInt")
                        nc.sync.dma_start_transpose(
                            out=sInt[:], in_=sT[:, half, :, :, :])
                        if (b == 0 and cp == 0 and half == 0 and rep == 0
                                and "dbg_sInt" in taps):
                            nc.sync.dma_start(out=taps["dbg_sInt"][:],
                                              in_=sInt[:])

                        for ig in range(16):
                            psm = psB.tile([GRP, N], F32, tag="psm")
                            for jj in range(2):
                                nc.tensor.matmul(
                                    psm[:, jj * 512:(jj + 1) * 512],
                                    lhsT=bd_pre[:],
                                    rhs=sInt[0:GRP, 4 * jj:4 * jj + 4, ig, :],
                                    start=True, stop=True)
                            e_sb = work.tile([GRP, N], F32R, tag="esb")
                            ssum = small.tile([GRP, 1], F32, tag="ssum")
                            nc.scalar.activation(
                                e_sb[:], psm[:],
                                mybir.ActivationFunctionType.Exp,
                                accum_out=ssum[:])
                            recip = small.tile([GRP, 1], F32, tag="recip")
                            nc.vector.reciprocal(recip[:], ssum[:])
                            bd_ps = small.tile([GRP, GRP], F32R, tag="bdps")
                            nc.vector.tensor_scalar_mul(bd_ps[:], bd_pat[:],
                                                        recip[:])
                            if (b == 0 and cp == 0 and half == 0 and ig == 0
                                    and rep == 0 and "dbg_E" in taps):
                                nc.sync.dma_start(out=taps["dbg_E"][:],
                                                  in_=e_sb[:].bitcast(F32))
                            psp = psB.tile([GRP, N], F32, tag="psp")
                            for jj in range(2):
                                nc.tensor.matmul(
                                    psp[:, jj * 512:(jj + 1) * 512],
                                    lhsT=bd_ps[:],
                                    rhs=e_sb[:, jj * 512:(jj + 1) * 512],
                                    start=True, stop=True)
                            if ig % 2 == 0:
                                p2 = work.tile([GRP, 2, N], FP16, tag="p2",
                                               name="p2")
                            nc.vector.tensor_copy(p2[:, ig % 2, :], psp[:])
                            if ig % 2 == 1:
                                igg = half * 16 + ig - 1
                                nc.sync.dma_start_transpose(
                                    out=pT[:, igg:igg + 2, :, :], in_=p2[:])

                    if b == 0 and cp == 0 and rep == 0 and "dbg_pT" in taps:
                        nc.sync.dma_start(out=taps["dbg_pT"][:], in_=pT[:])

                    # -------- attn @ V (free dim 256 = chunk pair) --------
                    oaT = work.tile([128, 6, 256], FP16, tag="oaT")
                    for r in range(3):
                        if r == 0:
                            av = psA.tile([128, 3, 512], F32, tag="A",
                                          name="av")
                        for q2 in range(2):
                            for hh in range(2):
                                h = 4 * r + 2 * q2 + hh
                                for jb in range(NJB):
                                    nc.tensor.matmul(
                                        av[64 * hh:64 * (hh + 1), r,
                                           256 * q2:256 * q2 + 256],
                                        lhsT=v_t[:, jb, h * 64:(h + 1) * 64],
                                        rhs=pT[:, :, jb, 8 * h:8 * h + 8],
                                        start=(q2 == 0 and jb == 0),
                                        stop=(q2 == 1 and jb == NJB - 1),
                                        skip_group_check=True,
                                        tile_position=(0, 64 * hh))
                        nc.vector.tensor_copy(
                            oaT[:, 2 * r:2 * r + 2, :],
                            av[:, r, :].rearrange("p (q c) -> p q c", q=2))

                    if (b == 0 and cp == NCP - 1 and rep == 0
                            and "dbg_oaT" in taps):
                        nc.sync.dma_start(out=taps["dbg_oaT"][:], in_=oaT[:])

                    # -------- output projection + bias --------
                    y_sb = work.tile([128, 2, DIM], FP16, tag="ysb")
                    for c in range(2):
                        psy = psA.tile([128, 2, 512], F32, tag="A")
                        for kt in range(NK):
                            for nh in range(2):
                                nc.tensor.matmul(
                                    psy[:, nh, 0:384],
                                    lhsT=oaT[:, kt, c * 128:(c + 1) * 128],
                                    rhs=wo_t[:, kt, nh * 384:nh * 384 + 384],
                                    start=(kt == 0), stop=False)
                        for nh in range(2):
                            nc.tensor.matmul(
                                psy[:, nh, 0:384], lhsT=ones_t[:],
                                rhs=bo_t[:, nh * 384:nh * 384 + 384],
                                start=False, stop=True)
                        nc.vector.tensor_copy(
                            y_sb[:, c, :].rearrange("p (a c) -> p a c", a=2),
                            psy[:, :, 0:384])
                    nc.sync.dma_start(
                        out=y_d[b, cp * 256:(cp + 1) * 256, :]
                        .rearrange("(c p) d -> p c d", p=128),
                        in_=y_sb[:])

    nc.compile()
    return nc


def host_prep(inputs):
    """Pack weights fp16 (scale folded into Wq) + block mix matrices."""
    mix_pre = np.asarray(inputs["mix_pre"], dtype=np.float32)
    mix_post = np.asarray(inputs["mix_post"], dtype=np.float32)
    bd_pre = np.zeros((GRP, GRP), dtype=np.float32)
    bd_pat = np.zeros((GRP, GRP), dtype=np.float32)
    for i in range(ILOC):
        bd_pre[H * i:H * i + H, H * i:H * i + H] = mix_pre
        for h in range(H):
            bd_pat[H * i:H * i + H, h * ILOC + i] = mix_post[:, h]
    wq = np.asarray(inputs["Wq"], dtype=np.float32) * SCALE
    wkv = np.asarray(inputs["Wkv"], dtype=np.float32)
    w3 = np.concatenate([wq, wkv], axis=1)  # [768, 2304]
    common = {
        "w3b": w3.astype(FP16_NP),
        "wob": np.asarray(inputs["Wo"], dtype=np.float32).astype(FP16_NP),
        "bob": np.asarray(inputs["bo"], dtype=np.float32).reshape(1, DIM)
               .astype(FP16_NP),
        "bdpre": bd_pre.astype(FP16_NP),
        "bdpat": bd_pat,
    }
    return common


def kernel(**inputs):
    x = np.asarray(inputs["x"], dtype=np.float32).astype(FP16_NP)
    common = host_prep(inputs)
    nc = build_program()
    in_maps = []
    for c in range(N_CORES):
        m = dict(common)
        m["xb"] = np.ascontiguousarray(x[c * B:(c + 1) * B])
        in_maps.append(m)
    res = run_bass_kernel_spmd(nc, in_maps, list(range(N_CORES)))
    out = np.concatenate([res.results[c]["y"] for c in range(N_CORES)], axis=0)
    return out.astype(np.float32)


if __name__ == "__main__":
    rng = np.random.default_rng(0)
    ins = {
        "x": rng.standard_normal((B_TOTAL, N, DIM), dtype=np.float32),
        "Wq": rng.standard_normal((DIM, HC), dtype=np.float32) * DIM ** -0.5,
        "Wkv": rng.standard_normal((DIM, 2 * HC), dtype=np.float32) * DIM ** -0.5,
        "mix_pre": rng.standard_normal((H, H), dtype=np.float32),
        "mix_post": rng.standard_normal((H, H), dtype=np.float32),
        "Wo": rng.standard_normal((HC, DIM), dtype=np.float32) * HC ** -0.5,
        "bo": np.zeros(DIM, dtype=np.float32),
    }
    y = kernel(**ins)
    print("kernel output", y.shape, y.dtype, float(np.abs(y).max()))
